# revision 6
# baseline (speedup 1.0000x reference)
"""GAT 2-layer kernel for 8 TRN2 NeuronCores.

Strategy (edge-parallel per sharding hint): destination nodes are split
into 8 contiguous slices (6250/core). Each core owns all edges into its
slice. Edges are sorted by dst, grouped into 128-node dst-blocks, padded
into a uniform [NBLK x TB] grid of 128-edge tiles (identical program on
all cores; per-core data differs only in inputs).

Per layer, per edge tile:
  - batched indirect DMA gathers node-table rows by src (h | alpha_src)
    and by dst (alpha_dst)
  - w = exp(leaky_relu(asrc[src]+adst[dst]))  (max subtraction is not
    needed: exponents are O(1); exp(e-m)/sum == exp(e)/sum exactly)
  - selection matrix S[e,n] = (dst_local[e]==n) via is_equal
  - PSUM-accumulated matmuls: acc += S^T @ (w*h),  s += S^T @ w
  - per block: out = acc/(s+eps) + bias (+ELU and the layer-2 node
    projection fused for layer 1)

Two launches: kernel1 = h1/alpha table build (replicated) + layer-1 edge
phase + fused [h2|a_src2|a_dst2] projection; host concatenates the 8
node-slice outputs into the full layer-2 table; kernel2 = layer-2 edge
phase + bias. This avoids mid-kernel collectives.
"""
import sys

sys.path.insert(0, '/opt/trn_rl_repo')

import numpy as np

import concourse.bass as bass
import concourse.bacc as bacc
import concourse.mybir as mybir
import concourse.tile as tile
from concourse.bass_utils import run_bass_kernel_spmd
from concourse.vector_clock import ScopedClock

f32 = mybir.dt.float32
i32 = mybir.dt.int32
P = 128
NCORES = 8
NEG_SLOPE = 0.2
EPS = 1e-16
HEADS1, OUT1 = 4, 32
HEADS2, OUT2 = 1, 32
G = 16  # tiles per gather group

_MAX_WAITS = 1


def _split_excess_waits(nc, max_waits=_MAX_WAITS):
    # this walrus build rejects >1 sem-wait per instruction; hoist excess
    # waits onto same-engine nops inserted right before the instruction
    for bb in nc.main_func.blocks:
        lst = bb.instructions
        out = []
        for inst in lst:
            si = inst.sync_info
            waits = list(si.on_wait) if si is not None and si.on_wait else []
            if len(waits) > max_waits:
                excess, keep = waits[:-max_waits], waits[-max_waits:]
                for w in excess:
                    nop = mybir.InstNoOp(
                        name=nc.get_next_instruction_name(), ins=[], outs=[]
                    )
                    nop.engine = inst.engine
                    nop.sync_info = mybir.SyncInfo(on_wait=[w], on_update=[])
                    nc.register_instruction(nop)
                    out.append(nop)
                si.on_wait.clear()
                for w in keep:
                    si.on_wait.append(w)
            out.append(inst)
        lst.clear()
        lst.extend(out)


def _patched_drain_and_barrier(self, tick_clock, wait_clock):
    nc = self.nc
    drain_inst = nc.sync.drain()
    wait_clock.add_sem_waits(
        drain_inst.ins, ScopedClock({None: tick_clock.global_clock})
    )
    nc.all_engine_barrier()
    assert self.sems is not None
    popped = nc._tile_sem_poison_stack.pop()
    assert popped is self._sem_poison
    nc.clear_and_free_semaphores(list(self.sems.allocated().values()))
    nc.all_engine_barrier()


tile.TileContext._drain_and_barrier = _patched_drain_and_barrier


def _edge_phase(nc, tc, pools, table, rowlen, fdim, nheads, srcg, dstg, dlocg,
                iota_t, ngroups, tb, nblk, out_cb):
    """Shared edge-aggregation phase (per-tile row gathers).

    table rows: [feat(fdim) | asrc(nheads) | adst(nheads)], rowlen f32.
    Grid: ngroups groups x G tiles; every TB tiles complete one dst block.
    out_cb(b, acc_psum, s_psum) consumes each finished block.
    """
    pool, psum = pools
    hc = fdim // nheads  # channels per head
    blk = 0
    acc = s_acc = None
    for q in range(ngroups):
        ix_s = pool.tile([P, G], i32, tag="ixs")
        nc.sync.dma_start(out=ix_s[:], in_=srcg[q])
        ix_d = pool.tile([P, G], i32, tag="ixd")
        nc.sync.dma_start(out=ix_d[:], in_=dstg[q])
        dloc = pool.tile([P, G], f32, tag="dloc")
        nc.sync.dma_start(out=dloc[:], in_=dlocg[q])

        for t in range(G):
            gtile = q * G + t
            if gtile >= nblk * tb:
                continue
            tt = gtile % tb  # position within block
            if tt == 0:
                acc = psum.tile([P, fdim], f32, space="PSUM", tag="acc")
                s_acc = psum.tile([P, nheads], f32, space="PSUM", tag="sacc")
            # per-tile row gathers (one row per partition per instruction)
            gs = pool.tile([P, rowlen], f32, tag="gs")
            nc.gpsimd.indirect_dma_start(
                out=gs[:], out_offset=None, in_=table[:],
                in_offset=bass.IndirectOffsetOnAxis(ap=ix_s[:, t:t + 1], axis=0))
            gd = pool.tile([P, rowlen], f32, tag="gd")
            nc.gpsimd.indirect_dma_start(
                out=gd[:], out_offset=None, in_=table[:],
                in_offset=bass.IndirectOffsetOnAxis(ap=ix_d[:, t:t + 1], axis=0))
            # w = exp(lrelu(asrc[src] + adst[dst]))   [P, nheads]
            w_t = pool.tile([P, nheads], f32, tag="w")
            nc.vector.tensor_tensor(
                out=w_t[:], in0=gs[:, fdim:fdim + nheads],
                in1=gd[:, fdim + nheads:fdim + 2 * nheads],
                op=mybir.AluOpType.add)
            lr_t = pool.tile([P, nheads], f32, tag="lr")
            nc.vector.tensor_scalar(out=lr_t[:], in0=w_t[:], scalar1=NEG_SLOPE,
                                    scalar2=None, op0=mybir.AluOpType.mult)
            nc.vector.tensor_tensor(out=w_t[:], in0=w_t[:], in1=lr_t[:],
                                    op=mybir.AluOpType.max)
            nc.scalar.activation(w_t[:], w_t[:],
                                 mybir.ActivationFunctionType.Exp)
            # S[e, n] = (dst_local[e] == n)
            s_t = pool.tile([P, P], f32, tag="st")
            nc.vector.tensor_scalar(
                out=s_t[:], in0=iota_t[:], scalar1=dloc[:, t:t + 1], scalar2=None,
                op0=mybir.AluOpType.is_equal)
            # M = h * w (per-head broadcast over channels)
            m_t = pool.tile([P, fdim], f32, tag="mt")
            w_ap = bass.AP(w_t[:].tensor, w_t[:].offset,
                           [w_t[:].ap[0], [1, nheads], [0, hc]])
            nc.vector.tensor_tensor(
                out=m_t[:].rearrange("p (h c) -> p h c", c=hc),
                in0=gs[:, 0:fdim].rearrange("p (h c) -> p h c", c=hc),
                in1=w_ap, op=mybir.AluOpType.mult)
            first, last = (tt == 0), (tt == tb - 1)
            nc.tensor.matmul(acc[:], lhsT=s_t[:], rhs=m_t[:],
                             start=first, stop=last)
            nc.tensor.matmul(s_acc[:], lhsT=s_t[:], rhs=w_t[:],
                             start=first, stop=last)
            if last:
                out_cb(blk, acc, s_acc)
                blk += 1


def _build_kernel1(NB, TB, NGRP, N, F_IN, F1):
    ROW1 = F_IN + 2 * HEADS1  # 136: [h1 | asrc1 | adst1]
    NT0 = (N + P - 1) // P
    nc = bacc.Bacc(None, target_bir_lowering=False)
    x = nc.dram_tensor("x", [N, F_IN], f32, kind="ExternalInput")
    w1cat = nc.dram_tensor("w1cat", [F_IN, ROW1], f32, kind="ExternalInput")
    w2cat = nc.dram_tensor("w2cat", [F1, OUT2 + 2], f32, kind="ExternalInput")
    b1t = nc.dram_tensor("b1t", [P, F1], f32, kind="ExternalInput")
    ident = nc.dram_tensor("ident", [P, P], f32, kind="ExternalInput")
    iota = nc.dram_tensor("iota", [P, P], f32, kind="ExternalInput")
    srcg = nc.dram_tensor("srcg", [NGRP, P, G], i32, kind="ExternalInput")
    dstg = nc.dram_tensor("dstg", [NGRP, P, G], i32, kind="ExternalInput")
    dlocg = nc.dram_tensor("dlocg", [NGRP, P, G], f32, kind="ExternalInput")
    t3out = nc.dram_tensor("t3out", [NB * P, OUT2 + 2], f32, kind="ExternalOutput")
    t12 = nc.dram_tensor("t12", [NT0 * P, ROW1], f32)

    with tile.TileContext(nc) as tc:
        with (
            tc.tile_pool(name="const", bufs=1) as cpool,
            tc.tile_pool(name="sbuf", bufs=3) as pool,
            tc.tile_pool(name="psum", bufs=2, space="PSUM") as psum,
        ):
            ident_t = cpool.tile([P, P], f32)
            nc.sync.dma_start(out=ident_t[:], in_=ident[:])
            iota_t = cpool.tile([P, P], f32)
            nc.sync.dma_start(out=iota_t[:], in_=iota[:])
            w1_t = cpool.tile([F_IN, ROW1], f32)
            nc.sync.dma_start(out=w1_t[:], in_=w1cat[:])
            w2_t = cpool.tile([F1, OUT2 + 2], f32)
            nc.sync.dma_start(out=w2_t[:], in_=w2cat[:])
            b1_t = cpool.tile([P, F1], f32)
            nc.sync.dma_start(out=b1_t[:], in_=b1t[:])

            # ---- phase 0 (replicated): t12[n] = [x@W1 | x@Psrc | x@Pdst]
            for i in range(NT0):
                xt = pool.tile([P, F_IN], f32, tag="xt")
                nrow = min(P, N - i * P)
                if nrow < P:
                    nc.vector.memset(xt[:], 0.0)
                nc.sync.dma_start(out=xt[:nrow], in_=x[i * P:i * P + nrow, :])
                xT_ps = psum.tile([P, P], f32, space="PSUM", tag="xT", bufs=1)
                nc.tensor.transpose(out=xT_ps[:], in_=xt[:], identity=ident_t[:])
                xT = pool.tile([P, F_IN], f32, tag="xTs")
                nc.vector.tensor_copy(out=xT[:], in_=xT_ps[:])
                h_ps = psum.tile([P, ROW1], f32, space="PSUM", tag="hps", bufs=1)
                nc.tensor.matmul(h_ps[:], lhsT=xT[:], rhs=w1_t[:],
                                 start=True, stop=True)
                h_sb = pool.tile([P, ROW1], f32, tag="hsb")
                nc.vector.tensor_copy(out=h_sb[:], in_=h_ps[:])
                nc.sync.dma_start(out=t12[i * P:(i + 1) * P, :], in_=h_sb[:])

            tc.strict_bb_all_engine_barrier()

            # ---- layer-1 edge phase + fused epilogue
            def epi(b, acc, s_acc):
                r = pool.tile([P, HEADS1], f32, tag="r")
                nc.vector.tensor_scalar(out=r[:], in0=s_acc[:], scalar1=EPS,
                                        scalar2=None, op0=mybir.AluOpType.add)
                nc.vector.reciprocal(out=r[:], in_=r[:])
                o = pool.tile([P, F1], f32, tag="o")
                r_ap = bass.AP(r[:].tensor, r[:].offset,
                               [r[:].ap[0], [1, HEADS1], [0, OUT1]])
                nc.vector.tensor_tensor(
                    out=o[:].rearrange("p (h c) -> p h c", c=OUT1),
                    in0=acc[:].rearrange("p (h c) -> p h c", c=OUT1),
                    in1=r_ap, op=mybir.AluOpType.mult)
                nc.vector.tensor_tensor(out=o[:], in0=o[:], in1=b1_t[:],
                                        op=mybir.AluOpType.add)
                # elu(o) = max(o,0) + exp(min(o,0)) - 1
                mn = pool.tile([P, F1], f32, tag="mn")
                nc.vector.tensor_scalar(out=mn[:], in0=o[:], scalar1=0.0,
                                        scalar2=None, op0=mybir.AluOpType.min)
                nc.scalar.activation(mn[:], mn[:], mybir.ActivationFunctionType.Exp)
                mx = pool.tile([P, F1], f32, tag="mx")
                nc.vector.tensor_scalar(out=mx[:], in0=o[:], scalar1=0.0,
                                        scalar2=None, op0=mybir.AluOpType.max)
                nc.vector.tensor_tensor(out=o[:], in0=mn[:], in1=mx[:],
                                        op=mybir.AluOpType.add)
                nc.vector.tensor_scalar(out=o[:], in0=o[:], scalar1=-1.0,
                                        scalar2=None, op0=mybir.AluOpType.add)
                # project: t3 rows = elu_out1 @ [W2 | W2 a2s | W2 a2d]
                oT_ps = psum.tile([P, P], f32, space="PSUM", tag="oT", bufs=1)
                nc.tensor.transpose(out=oT_ps[:], in_=o[:], identity=ident_t[:])
                oT = pool.tile([P, F1], f32, tag="oTs")
                nc.vector.tensor_copy(out=oT[:], in_=oT_ps[:])
                t3_ps = psum.tile([P, OUT2 + 2], f32, space="PSUM", tag="t3p", bufs=1)
                nc.tensor.matmul(t3_ps[:], lhsT=oT[:], rhs=w2_t[:],
                                 start=True, stop=True)
                t3_sb = pool.tile([P, OUT2 + 2], f32, tag="t3s")
                nc.vector.tensor_copy(out=t3_sb[:], in_=t3_ps[:])
                nc.sync.dma_start(out=t3out[b * P:(b + 1) * P, :], in_=t3_sb[:])

            _edge_phase(nc, tc, (pool, psum), t12, ROW1, F1, HEADS1,
                        srcg, dstg, dlocg, iota_t, NGRP, TB, NB, epi)

    nc.compile()
    _split_excess_waits(nc)
    return nc


def _build_kernel2(NB, TB, NGRP, N):
    ROW2 = OUT2 + 2  # 34: [h2 | asrc2 | adst2]
    NT3 = (N + P - 1) // P
    nc = bacc.Bacc(None, target_bir_lowering=False)
    t3 = nc.dram_tensor("t3", [NT3 * P, ROW2], f32, kind="ExternalInput")
    b2t = nc.dram_tensor("b2t", [P, OUT2], f32, kind="ExternalInput")
    iota = nc.dram_tensor("iota", [P, P], f32, kind="ExternalInput")
    srcg = nc.dram_tensor("srcg", [NGRP, P, G], i32, kind="ExternalInput")
    dstg = nc.dram_tensor("dstg", [NGRP, P, G], i32, kind="ExternalInput")
    dlocg = nc.dram_tensor("dlocg", [NGRP, P, G], f32, kind="ExternalInput")
    oout = nc.dram_tensor("oout", [NB * P, OUT2], f32, kind="ExternalOutput")

    with tile.TileContext(nc) as tc:
        with (
            tc.tile_pool(name="const", bufs=1) as cpool,
            tc.tile_pool(name="sbuf", bufs=3) as pool,
            tc.tile_pool(name="psum", bufs=2, space="PSUM") as psum,
        ):
            iota_t = cpool.tile([P, P], f32)
            nc.sync.dma_start(out=iota_t[:], in_=iota[:])
            b2_t = cpool.tile([P, OUT2], f32)
            nc.sync.dma_start(out=b2_t[:], in_=b2t[:])

            def epi(b, acc, s_acc):
                r = pool.tile([P, 1], f32, tag="r")
                nc.vector.tensor_scalar(out=r[:], in0=s_acc[:], scalar1=EPS,
                                        scalar2=None, op0=mybir.AluOpType.add)
                nc.vector.reciprocal(out=r[:], in_=r[:])
                o = pool.tile([P, OUT2], f32, tag="o")
                nc.vector.tensor_tensor(out=o[:], in0=acc[:],
                                        in1=r[:, 0:1].to_broadcast([P, OUT2]),
                                        op=mybir.AluOpType.mult)
                nc.vector.tensor_tensor(out=o[:], in0=o[:], in1=b2_t[:],
                                        op=mybir.AluOpType.add)
                nc.sync.dma_start(out=oout[b * P:(b + 1) * P, :], in_=o[:])

            _edge_phase(nc, tc, (pool, psum), t3, ROW2, OUT2, HEADS2,
                        srcg, dstg, dlocg, iota_t, NGRP, TB, NB, epi)

    nc.compile()
    _split_excess_waits(nc)
    return nc


def _prep_edges(src, dst, N):
    """Per-core edge grids. Returns per-core dicts + grid dims."""
    npc = (N + NCORES - 1) // NCORES  # nodes per core
    NB = (npc + P - 1) // P           # dst blocks per core
    cores = []
    maxtiles = 0
    for k in range(NCORES):
        lo, hi = k * npc, min((k + 1) * npc, N)
        sel = (dst >= lo) & (dst < hi)
        s, d = src[sel], dst[sel] - lo
        order = np.argsort(d, kind='stable')
        s, d = s[order], d[order]
        blocks = []
        for b in range(NB):
            bs = (d >= b * P) & (d < (b + 1) * P)
            blocks.append((s[bs], d[bs] - b * P))
            maxtiles = max(maxtiles, (len(blocks[-1][0]) + P - 1) // P)
        cores.append(blocks)
    TB = max(maxtiles, 1)
    ntiles = NB * TB
    NGRP = (ntiles + G - 1) // G
    ntiles_pad = NGRP * G
    out = []
    for k in range(NCORES):
        lo = k * npc
        srcg = np.zeros((ntiles_pad, P), np.int32)
        dstg = np.zeros((ntiles_pad, P), np.int32)
        dlocg = np.full((ntiles_pad, P), -1.0, np.float32)
        for b in range(NB):
            s, dl = cores[k][b]
            ne = len(s)
            t0 = b * TB
            srcg[t0:t0 + TB].reshape(-1)[:ne] = s
            dstg[t0:t0 + TB].reshape(-1)[:ne] = np.minimum(dl + b * P + lo, N - 1)
            dlocg[t0:t0 + TB].reshape(-1)[:ne] = dl.astype(np.float32)
        # device layout: [group, partition, g] with edge slot (tile, p)
        def to_grid(a):
            return np.ascontiguousarray(
                a.reshape(NGRP, G, P).transpose(0, 2, 1))
        out.append({"srcg": to_grid(srcg), "dstg": to_grid(dstg),
                    "dlocg": to_grid(dlocg)})
    return out, NB, TB, NGRP


def kernel(x, edge_index, W1, a_src1, a_dst1, b1, W2, a_src2, a_dst2, b2):
    x = np.asarray(x, np.float32)
    N, F_IN = x.shape
    F1 = HEADS1 * OUT1
    E = edge_index.shape[1]
    loops = np.arange(N, dtype=np.int64)
    src = np.concatenate([np.asarray(edge_index[0], np.int64), loops])
    dst = np.concatenate([np.asarray(edge_index[1], np.int64), loops])

    grids, NB, TB, NGRP = _prep_edges(src, dst, N)
    npc = (N + NCORES - 1) // NCORES

    # host-side weight prep (weights only -- no activations computed here)
    W1 = np.asarray(W1, np.float32)
    A1s = np.zeros((F1, HEADS1), np.float32)
    A1d = np.zeros((F1, HEADS1), np.float32)
    for h in range(HEADS1):
        A1s[h * OUT1:(h + 1) * OUT1, h] = np.asarray(a_src1, np.float32)[h]
        A1d[h * OUT1:(h + 1) * OUT1, h] = np.asarray(a_dst1, np.float32)[h]
    w1cat = np.concatenate([W1, W1 @ A1s, W1 @ A1d], axis=1)  # [F_IN, 136]
    W2 = np.asarray(W2, np.float32)
    w2cat = np.concatenate(
        [W2, W2 @ np.asarray(a_src2, np.float32).reshape(OUT2, 1),
         W2 @ np.asarray(a_dst2, np.float32).reshape(OUT2, 1)], axis=1)
    b1t = np.tile(np.asarray(b1, np.float32)[None, :], (P, 1))
    b2t = np.tile(np.asarray(b2, np.float32)[None, :], (P, 1))
    ident = np.eye(P, dtype=np.float32)
    iota = np.tile(np.arange(P, dtype=np.float32)[None, :], (P, 1))

    nc1 = _build_kernel1(NB, TB, NGRP, N, F_IN, F1)
    ins1 = [{"x": x, "w1cat": w1cat, "w2cat": w2cat, "b1t": b1t,
             "ident": ident, "iota": iota, **grids[k]} for k in range(NCORES)]
    res1 = run_bass_kernel_spmd(nc1, ins1, core_ids=list(range(NCORES)))

    # assemble full layer-2 node table from per-core slices
    NT3 = (N + P - 1) // P
    t3 = np.zeros((NT3 * P, OUT2 + 2), np.float32)
    for k in range(NCORES):
        lo, hi = k * npc, min((k + 1) * npc, N)
        t3[lo:hi] = res1.results[k]["t3out"][:hi - lo]

    nc2 = _build_kernel2(NB, TB, NGRP, N)
    ins2 = [{"t3": t3, "b2t": b2t, "iota": iota, **grids[k]}
            for k in range(NCORES)]
    res2 = run_bass_kernel_spmd(nc2, ins2, core_ids=list(range(NCORES)))

    out = np.zeros((N, OUT2), np.float32)
    for k in range(NCORES):
        lo, hi = k * npc, min((k + 1) * npc, N)
        out[lo:hi] = res2.results[k]["oout"][:hi - lo]
    return out
